# revision 43
# baseline (speedup 1.0000x reference)
"""Block-diagonal MLP kernel for Trainium2 (8 NeuronCores, expert-sharded).

Computes out = blockdiag_matmul(x, weights) + bias where
  x: [4, 2048, 4096] f32, weights: [32, 128, 128] f32, bias: [4096] f32.

Strategy: shard the 32 diagonal blocks across 8 cores (4 blocks = 512
feature columns each); every core sees all 8192 flattened rows of its
512-column slice.  Per-core DMA is 16.78 MB in + 16.78 MB out + 0.2 MB
consts.  Measured per-core HBM read+write tops out at ~430 GB/s, so
the body floor is ~78 us; the schedule's whole job is to keep loads
and stores co-flowing at that rate with no solo phases (a single HWDGE
ring caps at ~240-340 GB/s).

The host packs each core's x shard as [128, 32768] (partition p holds
the rows congruent to p mod 128, 64 row-groups side by side), so DMA
per-partition lines are 8 KiB (2 KiB descriptor lines measured ~35%
slower per ring).  x streams through a rotating pool of [128, 4096]
chunk buffers (8 groups each) on the SWDGE (gpsimd) queue, which casts
f32 DRAM -> bf16 SBUF inline; that frees BOTH HWDGE rings for stores,
which alternate per out-tile (8.4 MB each ring).  Loads self-pace to
compute rate via chunk-buffer reuse, and stores (ready from ~14 us)
overlap loads for the whole body.  The last two out-tiles store
per-pair alternating across both rings so the tail only waits on
512 KiB.

Per 512-column group: PE transpose-mode matmuls (bf16) put the
contraction dim on partitions; ACT evacuates the transpose to SBUF;
bf16 matmuls against SBUF-resident bf16 weights (host-cast, the same
4 blocks for all 64 groups); DVE evacuates with the bias add fused.
Transposes run two groups ahead of the consuming matmuls.  The bias
[1,512] row is broadcast to 128 partitions once on-chip via a K=1
ones-matmul.  bf16 is only used for matmul operands (fp32 PSUM
accumulation); max rel err vs the fp32 reference ~2e-3 (gate 2e-2).
"""
import numpy as np
import ml_dtypes
from contextlib import ExitStack

import concourse.mybir as mybir
import concourse.tile as tile
from concourse import bacc
from concourse.bass_utils import run_bass_kernel_spmd

F32 = mybir.dt.float32
BF16 = mybir.dt.bfloat16

SIZE = 4096
NB = 32            # number of diagonal blocks
BLK = 128          # block size
N_CORES = 8
NB_CORE = NB // N_CORES        # 4 blocks per core
C_CORE = NB_CORE * BLK         # 512 feature columns per core
B_FULL = 4 * 2048              # 8192 flattened rows (all on every core)
GROUPS = B_FULL // 128         # 64 row-groups of [128, 512]
XP_COLS = GROUPS * C_CORE      # 32768 packed columns
G_PER_CHUNK = 8                # groups per load chunk [128, 4096]
N_CHUNKS = GROUPS // G_PER_CHUNK
G_PER_OUT = 4                  # groups per store tile [128, 2048]
TAIL_GROUPS = 8                # last groups stored per-group on both rings

_NC_CACHE = {}


def _build_nc():
    nc = bacc.Bacc()
    x_d = nc.declare_dram_parameter("x", [128, XP_COLS], F32, isOutput=False)
    w_d = nc.declare_dram_parameter("weights", [BLK, C_CORE], BF16, isOutput=False)
    b_d = nc.declare_dram_parameter("bias", [1, C_CORE], F32, isOutput=False)
    i_d = nc.declare_dram_parameter("ident", [BLK, BLK], BF16, isOutput=False)
    if32_d = nc.declare_dram_parameter("ident32", [BLK, BLK], F32, isOutput=False)
    n_d = nc.declare_dram_parameter("ones", [1, BLK], F32, isOutput=False)
    o_d = nc.declare_dram_parameter("out", [128, XP_COLS], F32, isOutput=True)

    with tile.TileContext(nc) as tc, ExitStack() as ctx:
        consts = ctx.enter_context(tc.tile_pool(name="consts", bufs=1))
        x_pool = ctx.enter_context(tc.tile_pool(name="x", bufs=3))
        xh_pool = ctx.enter_context(tc.tile_pool(name="xh", bufs=5))
        xt_pool = ctx.enter_context(tc.tile_pool(name="xt", bufs=4))
        out_pool = ctx.enter_context(tc.tile_pool(name="out", bufs=4))
        tp_pool = ctx.enter_context(tc.tile_pool(name="tp", bufs=3, space="PSUM"))
        mp_pool = ctx.enter_context(tc.tile_pool(name="mp", bufs=3, space="PSUM"))
        bp_pool = ctx.enter_context(tc.tile_pool(name="bp", bufs=1, space="PSUM"))

        ident = consts.tile([BLK, BLK], BF16)
        ident32 = consts.tile([BLK, BLK], F32)
        ones = consts.tile([1, BLK], F32)
        w_sb = consts.tile([BLK, C_CORE], BF16)
        b_row = consts.tile([1, C_CORE], F32)
        bias_sb = consts.tile([128, C_CORE], F32)

        # Consts: identity (needed by the first transpose ~10.5 us in)
        # leads the Sync ring; weights/bias lead the ACT ring ahead of
        # the x stream.
        nc.sync.dma_start(out=ident, in_=i_d[:, :])
        nc.sync.dma_start(out=ident32, in_=if32_d[:, :])
        nc.sync.dma_start(out=ones, in_=n_d[:, :])
        nc.scalar.dma_start(out=w_sb, in_=w_d[:, :])
        nc.scalar.dma_start(out=b_row, in_=b_d[:, :])

        # Broadcast bias across partitions: [128,512] = ones.T @ b_row.
        bp = bp_pool.tile([128, C_CORE], F32)
        nc.tensor.matmul(bp, ones, b_row, start=True, stop=True)
        nc.vector.tensor_copy(bias_sb, bp)

        x_chunks = [None] * N_CHUNKS
        # Head (groups 0-15): the SWDGE-cast stream alone only delivers
        # ~200 GB/s while both HWDGE rings idle until the first stores,
        # so the first two chunks' worth of x is pulled by ALL THREE
        # queues in parallel -- group 0 via SWDGE (first bytes), groups
        # 1-15 as f32 pieces split across both rings -- and SWDGE jumps
        # ahead to chunk 2.  head_bufs[g] = (tile, col offset, is_f32).
        head_bufs = [None] * 16

        def emit_load(c):
            # SWDGE (gpsimd) DMA casts f32 DRAM -> bf16 SBUF inline in
            # the DMA engines: frees both HWDGE rings for stores.
            xc = x_pool.tile([128, G_PER_CHUNK * C_CORE], BF16)
            base = c * G_PER_CHUNK * C_CORE
            nc.gpsimd.dma_start(out=xc, in_=x_d[:, base:base + G_PER_CHUNK * C_CORE])
            x_chunks[c] = xc

        xhb = xh_pool.tile([128, 512], BF16)
        nc.gpsimd.dma_start(out=xhb, in_=x_d[:, 0:512])
        head_bufs[0] = (xhb, 0, False)
        for piece, (lo, hi, eng) in enumerate([
            (512, 2048, nc.scalar),
            (2048, 4096, nc.sync),
            (4096, 6144, nc.scalar),
            (6144, 8192, nc.sync),
        ]):
            xf = xh_pool.tile([128, hi - lo], F32)
            eng.dma_start(out=xf, in_=x_d[:, lo:hi])
            for g in range(lo // 512, hi // 512):
                head_bufs[g] = (xf, (g - lo // 512) * C_CORE, True)
        emit_load(2)
        emit_load(3)

        def emit_transposes(g):
            if g < 16:
                xc, gb, f32_src = head_bufs[g]
            else:
                xc = x_chunks[g // G_PER_CHUNK]
                gb = (g % G_PER_CHUNK) * C_CORE
                f32_src = False
            tp = tp_pool.tile([128, C_CORE], F32 if f32_src else BF16)
            idn = ident32 if f32_src else ident
            for j in range(NB_CORE):
                nc.tensor.matmul(
                    tp[:, j * 128:(j + 1) * 128],
                    xc[:, gb + j * 128:gb + (j + 1) * 128],
                    idn,
                    is_transpose=True,
                    start=(j == 0),
                    stop=(j == NB_CORE - 1),
                )
            xt = xt_pool.tile([128, C_CORE], BF16)
            nc.scalar.copy(xt, tp)   # PSUM -> SBUF bf16 (casts f32 head pieces)
            return xt

        xt_q = [emit_transposes(0), emit_transposes(1)]
        out_tile = None
        for g in range(GROUPS):
            if g % G_PER_OUT == 0:
                out_tile = out_pool.tile([128, G_PER_OUT * C_CORE], F32)
            # prefetch (chunks 2,3 preloaded; 4+ paced by chunk-buffer WAR)
            if g % G_PER_CHUNK == 0 and 4 <= (gc := g // G_PER_CHUNK + 4) < N_CHUNKS:
                emit_load(gc)
            xt = xt_q.pop(0)
            if g + 2 < GROUPS:
                xt_q.append(emit_transposes(g + 2))
            mp = mp_pool.tile([128, C_CORE], F32)
            for j in range(NB_CORE):
                nc.tensor.matmul(
                    mp[:, j * 128:(j + 1) * 128],
                    xt[:, j * 128:(j + 1) * 128],
                    w_sb[:, j * 128:(j + 1) * 128],
                    start=(j == 0),
                    stop=(j == NB_CORE - 1),
                )
            gi = (g % G_PER_OUT) * C_CORE
            nc.vector.tensor_add(out_tile[:, gi:gi + C_CORE], mp, bias_sb)
            # Loads ride the SWDGE queue, so BOTH HWDGE rings carry
            # stores: alternate out-tiles between them (8.4 MB each).
            if g >= GROUPS - TAIL_GROUPS:
                # tail: store per-pair alternating rings so the kernel
                # tail only waits on 512 KiB.
                if g % 2 == 1:
                    eng = nc.sync if g % 4 == 1 else nc.scalar
                    cols = slice((g - 1) * C_CORE, (g + 1) * C_CORE)
                    eng.dma_start(
                        out=o_d[:, cols],
                        in_=out_tile[:, gi - C_CORE:gi + C_CORE],
                    )
            elif g % G_PER_OUT == G_PER_OUT - 1:
                t = g // G_PER_OUT
                eng = nc.sync if t % 2 == 0 else nc.scalar
                cols = slice(t * G_PER_OUT * C_CORE, (t + 1) * G_PER_OUT * C_CORE)
                eng.dma_start(out=o_d[:, cols], in_=out_tile)

    nc.compile()
    return nc


def _get_nc():
    if "nc" not in _NC_CACHE:
        _NC_CACHE["nc"] = _build_nc()
    return _NC_CACHE["nc"]


def _run(inputs, trace=False):
    x = np.asarray(inputs["x"], dtype=np.float32)
    weights = np.asarray(inputs["weights"], dtype=np.float32)
    bias = np.asarray(inputs["bias"], dtype=np.float32)
    orig_shape = x.shape
    xf = x.reshape(B_FULL, SIZE)
    ident32 = np.eye(BLK, dtype=np.float32)
    ident = ident32.astype(ml_dtypes.bfloat16)
    ones = np.ones((1, BLK), dtype=np.float32)

    nc = _get_nc()
    in_maps = []
    for i in range(N_CORES):
        cols = slice(i * C_CORE, (i + 1) * C_CORE)
        # pack: xp[p, g*512 + c] = xf[g*128 + p, 512*i + c]
        xp = np.ascontiguousarray(
            xf[:, cols].reshape(GROUPS, 128, C_CORE).transpose(1, 0, 2)
            .reshape(128, XP_COLS)
        )
        # weights d-major per core: [d, j*128+e] = W[4i+j, d, e], cast bf16
        w_t = np.ascontiguousarray(
            weights[i * NB_CORE:(i + 1) * NB_CORE].transpose(1, 0, 2)
            .reshape(BLK, C_CORE)
        ).astype(ml_dtypes.bfloat16)
        in_maps.append(
            {
                "x": xp,
                "weights": w_t,
                "bias": np.ascontiguousarray(bias[cols][None, :]),
                "ident": ident,
                "ident32": ident32,
                "ones": ones,
            }
        )
    res = run_bass_kernel_spmd(
        nc, in_maps, core_ids=list(range(N_CORES)), trace=trace
    )
    out = np.empty((B_FULL, SIZE), dtype=np.float32)
    for i in range(N_CORES):
        cols = slice(i * C_CORE, (i + 1) * C_CORE)
        op = res.results[i]["out"]
        out[:, cols] = (
            op.reshape(128, GROUPS, C_CORE).transpose(1, 0, 2)
            .reshape(B_FULL, C_CORE)
        )
    return out.reshape(orig_shape), res


def kernel(**inputs):
    out, _ = _run(inputs, trace=False)
    return out


# revision 44
# speedup vs baseline: 1.0312x; 1.0312x over previous
"""Block-diagonal MLP kernel for Trainium2 (8 NeuronCores, expert-sharded).

Computes out = blockdiag_matmul(x, weights) + bias where
  x: [4, 2048, 4096] f32, weights: [32, 128, 128] f32, bias: [4096] f32.

Strategy: shard the 32 diagonal blocks across 8 cores (4 blocks = 512
feature columns each); every core sees all 8192 flattened rows of its
512-column slice.  Per-core DMA is 16.78 MB in + 16.78 MB out + 0.2 MB
consts.  Measured per-core HBM read+write tops out at ~430 GB/s, so
the body floor is ~78 us; the schedule's whole job is to keep loads
and stores co-flowing at that rate with no solo phases (a single HWDGE
ring caps at ~240-340 GB/s).

The host packs each core's x shard as [128, 32768] (partition p holds
the rows congruent to p mod 128, 64 row-groups side by side), so DMA
per-partition lines are 8 KiB (2 KiB descriptor lines measured ~35%
slower per ring).  x streams through a rotating pool of [128, 4096]
chunk buffers (8 groups each) on the SWDGE (gpsimd) queue, which casts
f32 DRAM -> bf16 SBUF inline; that frees BOTH HWDGE rings for stores,
which alternate per out-tile (8.4 MB each ring).  Loads self-pace to
compute rate via chunk-buffer reuse, and stores (ready from ~14 us)
overlap loads for the whole body.  The last two out-tiles store
per-pair alternating across both rings so the tail only waits on
512 KiB.

Per 512-column group: PE transpose-mode matmuls (bf16) put the
contraction dim on partitions; ACT evacuates the transpose to SBUF;
bf16 matmuls against SBUF-resident bf16 weights (host-cast, the same
4 blocks for all 64 groups); DVE evacuates with the bias add fused.
Transposes run two groups ahead of the consuming matmuls.  The bias
[1,512] row is broadcast to 128 partitions once on-chip via a K=1
ones-matmul.  bf16 is only used for matmul operands (fp32 PSUM
accumulation); max rel err vs the fp32 reference ~2e-3 (gate 2e-2).
"""
import numpy as np
import ml_dtypes
from contextlib import ExitStack

import concourse.mybir as mybir
import concourse.tile as tile
from concourse import bacc
from concourse.bass_utils import run_bass_kernel_spmd

F32 = mybir.dt.float32
BF16 = mybir.dt.bfloat16

SIZE = 4096
NB = 32            # number of diagonal blocks
BLK = 128          # block size
N_CORES = 8
NB_CORE = NB // N_CORES        # 4 blocks per core
C_CORE = NB_CORE * BLK         # 512 feature columns per core
B_FULL = 4 * 2048              # 8192 flattened rows (all on every core)
GROUPS = B_FULL // 128         # 64 row-groups of [128, 512]
XP_COLS = GROUPS * C_CORE      # 32768 packed columns
G_PER_CHUNK = 8                # groups per load chunk [128, 4096]
N_CHUNKS = GROUPS // G_PER_CHUNK
G_PER_OUT = 4                  # groups per store tile [128, 2048]
TAIL_GROUPS = 8                # last groups stored per-group on both rings

_NC_CACHE = {}


def _build_nc():
    nc = bacc.Bacc()
    x_d = nc.declare_dram_parameter("x", [128, XP_COLS], F32, isOutput=False)
    w_d = nc.declare_dram_parameter("weights", [BLK, C_CORE], BF16, isOutput=False)
    b_d = nc.declare_dram_parameter("bias", [1, C_CORE], F32, isOutput=False)
    i_d = nc.declare_dram_parameter("ident", [BLK, BLK], BF16, isOutput=False)
    n_d = nc.declare_dram_parameter("ones", [1, BLK], F32, isOutput=False)
    o_d = nc.declare_dram_parameter("out", [128, XP_COLS], F32, isOutput=True)

    with tile.TileContext(nc) as tc, ExitStack() as ctx:
        consts = ctx.enter_context(tc.tile_pool(name="consts", bufs=1))
        x_pool = ctx.enter_context(tc.tile_pool(name="x", bufs=3))
        xt_pool = ctx.enter_context(tc.tile_pool(name="xt", bufs=4))
        out_pool = ctx.enter_context(tc.tile_pool(name="out", bufs=4))
        tp_pool = ctx.enter_context(tc.tile_pool(name="tp", bufs=3, space="PSUM"))
        mp_pool = ctx.enter_context(tc.tile_pool(name="mp", bufs=3, space="PSUM"))
        bp_pool = ctx.enter_context(tc.tile_pool(name="bp", bufs=1, space="PSUM"))

        ident = consts.tile([BLK, BLK], BF16)
        ones = consts.tile([1, BLK], F32)
        w_sb = consts.tile([BLK, C_CORE], BF16)
        b_row = consts.tile([1, C_CORE], F32)
        bias_sb = consts.tile([128, C_CORE], F32)

        # Consts: identity (needed by the first transpose ~10.5 us in)
        # leads the Sync ring; weights/bias lead the ACT ring ahead of
        # the x stream.
        nc.sync.dma_start(out=ident, in_=i_d[:, :])
        nc.sync.dma_start(out=ones, in_=n_d[:, :])
        nc.scalar.dma_start(out=w_sb, in_=w_d[:, :])
        nc.scalar.dma_start(out=b_row, in_=b_d[:, :])

        # Broadcast bias across partitions: [128,512] = ones.T @ b_row.
        bp = bp_pool.tile([128, C_CORE], F32)
        nc.tensor.matmul(bp, ones, b_row, start=True, stop=True)
        nc.vector.tensor_copy(bias_sb, bp)

        x_chunks = [None] * N_CHUNKS

        def emit_load(c):
            # SWDGE (gpsimd) DMA casts f32 DRAM -> bf16 SBUF inline in the
            # DMA engines: halves the SBUF-fabric bytes on the load side
            # and halves the PE transpose cost, for free.
            xc = x_pool.tile([128, G_PER_CHUNK * C_CORE], BF16)
            base = c * G_PER_CHUNK * C_CORE
            if c == 0:
                # split so the first transposes start earlier
                nc.gpsimd.dma_start(out=xc[:, 0:512], in_=x_d[:, 0:512])
                nc.gpsimd.dma_start(out=xc[:, 512:2048], in_=x_d[:, 512:2048])
                nc.gpsimd.dma_start(out=xc[:, 2048:4096], in_=x_d[:, 2048:4096])
            else:
                nc.gpsimd.dma_start(out=xc, in_=x_d[:, base:base + G_PER_CHUNK * C_CORE])
            x_chunks[c] = xc

        for c in range(2):
            emit_load(c)

        def emit_transposes(g):
            tp = tp_pool.tile([128, C_CORE], BF16)
            xc = x_chunks[g // G_PER_CHUNK]
            gb = (g % G_PER_CHUNK) * C_CORE
            for j in range(NB_CORE):
                nc.tensor.matmul(
                    tp[:, j * 128:(j + 1) * 128],
                    xc[:, gb + j * 128:gb + (j + 1) * 128],
                    ident,
                    is_transpose=True,
                    start=(j == 0),
                    stop=(j == NB_CORE - 1),
                )
            xt = xt_pool.tile([128, C_CORE], BF16)
            nc.scalar.copy(xt, tp)   # PSUM f32 -> SBUF bf16
            return xt

        xt_q = [emit_transposes(0), emit_transposes(1)]
        out_tile = None
        for g in range(GROUPS):
            if g % G_PER_OUT == 0:
                out_tile = out_pool.tile([128, G_PER_OUT * C_CORE], F32)
            # prefetch: 2 chunks (16 groups) ahead of the transposes,
            # which themselves run 2 groups ahead of the matmuls here
            if g % G_PER_CHUNK == 0 and (gc := g // G_PER_CHUNK + 2) < N_CHUNKS:
                emit_load(gc)
            xt = xt_q.pop(0)
            if g + 2 < GROUPS:
                xt_q.append(emit_transposes(g + 2))
            mp = mp_pool.tile([128, C_CORE], F32)
            for j in range(NB_CORE):
                nc.tensor.matmul(
                    mp[:, j * 128:(j + 1) * 128],
                    xt[:, j * 128:(j + 1) * 128],
                    w_sb[:, j * 128:(j + 1) * 128],
                    start=(j == 0),
                    stop=(j == NB_CORE - 1),
                )
            gi = (g % G_PER_OUT) * C_CORE
            nc.vector.tensor_add(out_tile[:, gi:gi + C_CORE], mp, bias_sb)
            # Loads ride the SWDGE queue, so BOTH HWDGE rings carry
            # stores: alternate out-tiles between them (8.4 MB each).
            if g >= GROUPS - TAIL_GROUPS:
                # tail: store per-pair alternating rings so the kernel
                # tail only waits on 512 KiB.
                if g % 2 == 1:
                    eng = nc.sync if g % 4 == 1 else nc.scalar
                    cols = slice((g - 1) * C_CORE, (g + 1) * C_CORE)
                    eng.dma_start(
                        out=o_d[:, cols],
                        in_=out_tile[:, gi - C_CORE:gi + C_CORE],
                    )
            elif g % G_PER_OUT == G_PER_OUT - 1:
                t = g // G_PER_OUT
                eng = nc.sync if t % 2 == 0 else nc.scalar
                cols = slice(t * G_PER_OUT * C_CORE, (t + 1) * G_PER_OUT * C_CORE)
                eng.dma_start(out=o_d[:, cols], in_=out_tile)

    nc.compile()
    return nc


def _get_nc():
    if "nc" not in _NC_CACHE:
        _NC_CACHE["nc"] = _build_nc()
    return _NC_CACHE["nc"]


def _run(inputs, trace=False):
    x = np.asarray(inputs["x"], dtype=np.float32)
    weights = np.asarray(inputs["weights"], dtype=np.float32)
    bias = np.asarray(inputs["bias"], dtype=np.float32)
    orig_shape = x.shape
    xf = x.reshape(B_FULL, SIZE)
    ident = np.eye(BLK, dtype=np.float32).astype(ml_dtypes.bfloat16)
    ones = np.ones((1, BLK), dtype=np.float32)

    nc = _get_nc()
    in_maps = []
    for i in range(N_CORES):
        cols = slice(i * C_CORE, (i + 1) * C_CORE)
        # pack: xp[p, g*512 + c] = xf[g*128 + p, 512*i + c]
        xp = np.ascontiguousarray(
            xf[:, cols].reshape(GROUPS, 128, C_CORE).transpose(1, 0, 2)
            .reshape(128, XP_COLS)
        )
        # weights d-major per core: [d, j*128+e] = W[4i+j, d, e], cast bf16
        w_t = np.ascontiguousarray(
            weights[i * NB_CORE:(i + 1) * NB_CORE].transpose(1, 0, 2)
            .reshape(BLK, C_CORE)
        ).astype(ml_dtypes.bfloat16)
        in_maps.append(
            {
                "x": xp,
                "weights": w_t,
                "bias": np.ascontiguousarray(bias[cols][None, :]),
                "ident": ident,
                "ones": ones,
            }
        )
    res = run_bass_kernel_spmd(
        nc, in_maps, core_ids=list(range(N_CORES)), trace=trace
    )
    out = np.empty((B_FULL, SIZE), dtype=np.float32)
    for i in range(N_CORES):
        cols = slice(i * C_CORE, (i + 1) * C_CORE)
        op = res.results[i]["out"]
        out[:, cols] = (
            op.reshape(128, GROUPS, C_CORE).transpose(1, 0, 2)
            .reshape(B_FULL, C_CORE)
        )
    return out.reshape(orig_shape), res


def kernel(**inputs):
    out, _ = _run(inputs, trace=False)
    return out


# revision 47
# speedup vs baseline: 1.2424x; 1.2048x over previous
"""Block-diagonal MLP kernel for Trainium2 (8 NeuronCores, expert-sharded).

Computes out = blockdiag_matmul(x, weights) + bias where
  x: [4, 2048, 4096] f32, weights: [32, 128, 128] f32, bias: [4096] f32.

Strategy: shard the 32 diagonal blocks across 8 cores (4 blocks = 512
feature columns each); every core sees all 8192 flattened rows of its
512-column slice.  Per-core DMA is 16.78 MB in + 16.78 MB out + 0.2 MB
consts.  Measured per-core HBM read+write tops out at ~430 GB/s, so
the body floor is ~78 us; the schedule's whole job is to keep loads
and stores co-flowing at that rate with no solo phases (a single HWDGE
ring caps at ~240-340 GB/s).

The host packs each core's x shard as [128, 32768] (partition p holds
the rows congruent to p mod 128, 64 row-groups side by side), so DMA
per-partition lines are 8 KiB (2 KiB descriptor lines measured ~35%
slower per ring).  x streams through a rotating pool of [128, 4096]
chunk buffers (8 groups each) on the SWDGE (gpsimd) queue, which casts
f32 DRAM -> bf16 SBUF inline; that frees BOTH HWDGE rings for stores,
which alternate per out-tile (8.4 MB each ring).  Loads self-pace to
compute rate via chunk-buffer reuse, and stores (ready from ~14 us)
overlap loads for the whole body.  The last two out-tiles store
per-pair alternating across both rings so the tail only waits on
512 KiB.

Per 512-column group: PE transpose-mode matmuls (bf16) put the
contraction dim on partitions; ACT evacuates the transpose to SBUF;
bf16 matmuls against SBUF-resident bf16 weights (host-cast, the same
4 blocks for all 64 groups); DVE evacuates with the bias add fused.
Transposes run two groups ahead of the consuming matmuls.  The bias
[1,512] row is broadcast to 128 partitions once on-chip via a K=1
ones-matmul.  bf16 is only used for matmul operands (fp32 PSUM
accumulation); max rel err vs the fp32 reference ~2e-3 (gate 2e-2).
"""
import numpy as np
import ml_dtypes
from contextlib import ExitStack

import concourse.mybir as mybir
import concourse.tile as tile
from concourse import bacc
from concourse.bass_utils import run_bass_kernel_spmd

F32 = mybir.dt.float32
BF16 = mybir.dt.bfloat16

SIZE = 4096
NB = 32            # number of diagonal blocks
BLK = 128          # block size
N_CORES = 8
NB_CORE = NB // N_CORES        # 4 blocks per core
C_CORE = NB_CORE * BLK         # 512 feature columns per core
B_FULL = 4 * 2048              # 8192 flattened rows (all on every core)
GROUPS = B_FULL // 128         # 64 row-groups of [128, 512]
XP_COLS = GROUPS * C_CORE      # 32768 packed columns
G_PER_CHUNK = 8                # groups per load chunk [128, 4096]
N_CHUNKS = GROUPS // G_PER_CHUNK
G_PER_OUT = 4                  # groups per store tile [128, 2048]
TAIL_GROUPS = 8                # last groups stored per-group on both rings

_NC_CACHE = {}


def _build_nc():
    nc = bacc.Bacc()
    x_d = nc.declare_dram_parameter("x", [128, XP_COLS], BF16, isOutput=False)
    w_d = nc.declare_dram_parameter("weights", [BLK, C_CORE], BF16, isOutput=False)
    b_d = nc.declare_dram_parameter("bias", [1, C_CORE], F32, isOutput=False)
    i_d = nc.declare_dram_parameter("ident", [BLK, BLK], BF16, isOutput=False)
    n_d = nc.declare_dram_parameter("ones", [1, BLK], F32, isOutput=False)
    o_d = nc.declare_dram_parameter("out", [128, XP_COLS], F32, isOutput=True)

    with tile.TileContext(nc) as tc, ExitStack() as ctx:
        consts = ctx.enter_context(tc.tile_pool(name="consts", bufs=1))
        x_pool = ctx.enter_context(tc.tile_pool(name="x", bufs=3))
        xt_pool = ctx.enter_context(tc.tile_pool(name="xt", bufs=4))
        out_pool = ctx.enter_context(tc.tile_pool(name="out", bufs=4))
        tp_pool = ctx.enter_context(tc.tile_pool(name="tp", bufs=3, space="PSUM"))
        mp_pool = ctx.enter_context(tc.tile_pool(name="mp", bufs=3, space="PSUM"))
        bp_pool = ctx.enter_context(tc.tile_pool(name="bp", bufs=1, space="PSUM"))

        ident = consts.tile([BLK, BLK], BF16)
        ones = consts.tile([1, BLK], F32)
        w_sb = consts.tile([BLK, C_CORE], BF16)
        b_row = consts.tile([1, C_CORE], F32)
        bias_sb = consts.tile([128, C_CORE], F32)

        # Consts: identity (needed by the first transpose ~10.5 us in)
        # leads the Sync ring; weights/bias lead the ACT ring ahead of
        # the x stream.
        nc.sync.dma_start(out=ident, in_=i_d[:, :])
        nc.sync.dma_start(out=ones, in_=n_d[:, :])
        nc.scalar.dma_start(out=w_sb, in_=w_d[:, :])
        nc.scalar.dma_start(out=b_row, in_=b_d[:, :])

        # Broadcast bias across partitions: [128,512] = ones.T @ b_row.
        bp = bp_pool.tile([128, C_CORE], F32)
        nc.tensor.matmul(bp, ones, b_row, start=True, stop=True)
        nc.vector.tensor_copy(bias_sb, bp)

        x_chunks = [None] * N_CHUNKS

        def emit_load(c):
            # x is host-cast to bf16, halving load-side HBM bytes (the
            # dominant lever: per-core HBM drops 33.8 -> 25.4 MB).  Plain
            # SWDGE (gpsimd) loads keep both HWDGE rings free for stores.
            xc = x_pool.tile([128, G_PER_CHUNK * C_CORE], BF16)
            base = c * G_PER_CHUNK * C_CORE
            if c == 0:
                # split so the first transposes start earlier
                nc.gpsimd.dma_start(out=xc[:, 0:512], in_=x_d[:, 0:512])
                nc.gpsimd.dma_start(out=xc[:, 512:2048], in_=x_d[:, 512:2048])
                nc.gpsimd.dma_start(out=xc[:, 2048:4096], in_=x_d[:, 2048:4096])
            else:
                nc.gpsimd.dma_start(out=xc, in_=x_d[:, base:base + G_PER_CHUNK * C_CORE])
            x_chunks[c] = xc

        for c in range(2):
            emit_load(c)

        def emit_transposes(g):
            tp = tp_pool.tile([128, C_CORE], BF16)
            xc = x_chunks[g // G_PER_CHUNK]
            gb = (g % G_PER_CHUNK) * C_CORE
            for j in range(NB_CORE):
                nc.tensor.matmul(
                    tp[:, j * 128:(j + 1) * 128],
                    xc[:, gb + j * 128:gb + (j + 1) * 128],
                    ident,
                    is_transpose=True,
                    start=(j == 0),
                    stop=(j == NB_CORE - 1),
                )
            xt = xt_pool.tile([128, C_CORE], BF16)
            nc.scalar.copy(xt, tp)   # PSUM f32 -> SBUF bf16
            return xt

        xt_q = [emit_transposes(0), emit_transposes(1)]
        out_tile = None
        for g in range(GROUPS):
            if g % G_PER_OUT == 0:
                out_tile = out_pool.tile([128, G_PER_OUT * C_CORE], F32)
            # prefetch: 2 chunks (16 groups) ahead of the transposes,
            # which themselves run 2 groups ahead of the matmuls here
            if g % G_PER_CHUNK == 0 and (gc := g // G_PER_CHUNK + 2) < N_CHUNKS:
                emit_load(gc)
            xt = xt_q.pop(0)
            if g + 2 < GROUPS:
                xt_q.append(emit_transposes(g + 2))
            mp = mp_pool.tile([128, C_CORE], F32)
            for j in range(NB_CORE):
                nc.tensor.matmul(
                    mp[:, j * 128:(j + 1) * 128],
                    xt[:, j * 128:(j + 1) * 128],
                    w_sb[:, j * 128:(j + 1) * 128],
                    start=(j == 0),
                    stop=(j == NB_CORE - 1),
                )
            gi = (g % G_PER_OUT) * C_CORE
            nc.vector.tensor_add(out_tile[:, gi:gi + C_CORE], mp, bias_sb)
            # Loads ride the SWDGE queue, so BOTH HWDGE rings carry
            # stores: alternate out-tiles between them (8.4 MB each).
            if g >= GROUPS - TAIL_GROUPS:
                # tail: store per-pair alternating rings so the kernel
                # tail only waits on 512 KiB.
                if g % 2 == 1:
                    eng = nc.sync if g % 4 == 1 else nc.scalar
                    cols = slice((g - 1) * C_CORE, (g + 1) * C_CORE)
                    eng.dma_start(
                        out=o_d[:, cols],
                        in_=out_tile[:, gi - C_CORE:gi + C_CORE],
                    )
            elif g % G_PER_OUT == G_PER_OUT - 1:
                t = g // G_PER_OUT
                eng = nc.sync if t % 2 == 0 else nc.scalar
                cols = slice(t * G_PER_OUT * C_CORE, (t + 1) * G_PER_OUT * C_CORE)
                eng.dma_start(out=o_d[:, cols], in_=out_tile)

    nc.compile()
    return nc


def _get_nc():
    if "nc" not in _NC_CACHE:
        _NC_CACHE["nc"] = _build_nc()
    return _NC_CACHE["nc"]


def _run(inputs, trace=False):
    x = np.asarray(inputs["x"], dtype=np.float32)
    weights = np.asarray(inputs["weights"], dtype=np.float32)
    bias = np.asarray(inputs["bias"], dtype=np.float32)
    orig_shape = x.shape
    xf = x.reshape(B_FULL, SIZE)
    ident = np.eye(BLK, dtype=np.float32).astype(ml_dtypes.bfloat16)
    ones = np.ones((1, BLK), dtype=np.float32)

    nc = _get_nc()
    in_maps = []
    for i in range(N_CORES):
        cols = slice(i * C_CORE, (i + 1) * C_CORE)
        # pack: xp[p, g*512 + c] = xf[g*128 + p, 512*i + c]
        xp = np.ascontiguousarray(
            xf[:, cols].reshape(GROUPS, 128, C_CORE).transpose(1, 0, 2)
            .reshape(128, XP_COLS)
        ).astype(ml_dtypes.bfloat16)
        # weights d-major per core: [d, j*128+e] = W[4i+j, d, e], cast bf16
        w_t = np.ascontiguousarray(
            weights[i * NB_CORE:(i + 1) * NB_CORE].transpose(1, 0, 2)
            .reshape(BLK, C_CORE)
        ).astype(ml_dtypes.bfloat16)
        in_maps.append(
            {
                "x": xp,
                "weights": w_t,
                "bias": np.ascontiguousarray(bias[cols][None, :]),
                "ident": ident,
                "ones": ones,
            }
        )
    res = run_bass_kernel_spmd(
        nc, in_maps, core_ids=list(range(N_CORES)), trace=trace
    )
    out = np.empty((B_FULL, SIZE), dtype=np.float32)
    for i in range(N_CORES):
        cols = slice(i * C_CORE, (i + 1) * C_CORE)
        op = res.results[i]["out"]
        out[:, cols] = (
            op.reshape(128, GROUPS, C_CORE).transpose(1, 0, 2)
            .reshape(B_FULL, C_CORE)
        )
    return out.reshape(orig_shape), res


def kernel(**inputs):
    out, _ = _run(inputs, trace=False)
    return out


# revision 50
# speedup vs baseline: 1.2663x; 1.0192x over previous
"""Block-diagonal MLP kernel for Trainium2 (8 NeuronCores, expert-sharded).

Computes out = blockdiag_matmul(x, weights) + bias where
  x: [4, 2048, 4096] f32, weights: [32, 128, 128] f32, bias: [4096] f32.

Strategy: shard the 32 diagonal blocks across 8 cores (4 blocks = 512
feature columns each); every core sees all 8192 flattened rows of its
512-column slice.  Per-core DMA is 16.78 MB in + 16.78 MB out + 0.2 MB
consts.  Measured per-core HBM read+write tops out at ~430 GB/s, so
the body floor is ~78 us; the schedule's whole job is to keep loads
and stores co-flowing at that rate with no solo phases (a single HWDGE
ring caps at ~240-340 GB/s).

The host packs each core's x shard as [128, 32768] (partition p holds
the rows congruent to p mod 128, 64 row-groups side by side), so DMA
per-partition lines are 8 KiB (2 KiB descriptor lines measured ~35%
slower per ring).  x streams through a rotating pool of [128, 4096]
chunk buffers (8 groups each) on the SWDGE (gpsimd) queue, which casts
f32 DRAM -> bf16 SBUF inline; that frees BOTH HWDGE rings for stores,
which alternate per out-tile (8.4 MB each ring).  Loads self-pace to
compute rate via chunk-buffer reuse, and stores (ready from ~14 us)
overlap loads for the whole body.  The last two out-tiles store
per-pair alternating across both rings so the tail only waits on
512 KiB.

Per 512-column group: PE transpose-mode matmuls (bf16) put the
contraction dim on partitions; ACT evacuates the transpose to SBUF;
bf16 matmuls against SBUF-resident bf16 weights (host-cast, the same
4 blocks for all 64 groups); DVE evacuates with the bias add fused.
Transposes run two groups ahead of the consuming matmuls.  The bias
[1,512] row is broadcast to 128 partitions once on-chip via a K=1
ones-matmul.  bf16 is only used for matmul operands (fp32 PSUM
accumulation); max rel err vs the fp32 reference ~2e-3 (gate 2e-2).
"""
import numpy as np
import ml_dtypes
from contextlib import ExitStack

import concourse.mybir as mybir
import concourse.tile as tile
from concourse import bacc
from concourse.bass_utils import run_bass_kernel_spmd

F32 = mybir.dt.float32
BF16 = mybir.dt.bfloat16

SIZE = 4096
NB = 32            # number of diagonal blocks
BLK = 128          # block size
N_CORES = 8
NB_CORE = NB // N_CORES        # 4 blocks per core
C_CORE = NB_CORE * BLK         # 512 feature columns per core
B_FULL = 4 * 2048              # 8192 flattened rows (all on every core)
GROUPS = B_FULL // 128         # 64 row-groups of [128, 512]
XP_COLS = GROUPS * C_CORE      # 32768 packed columns
G_PER_CHUNK = 8                # groups per load chunk [128, 4096]
N_CHUNKS = GROUPS // G_PER_CHUNK
G_PER_OUT = 4                  # groups per store tile [128, 2048]
TAIL_GROUPS = 8                # last groups stored per-group on both rings

_NC_CACHE = {}


def _build_nc():
    nc = bacc.Bacc()
    x_d = nc.declare_dram_parameter("x", [128, XP_COLS], BF16, isOutput=False)
    w_d = nc.declare_dram_parameter("weights", [BLK, C_CORE], BF16, isOutput=False)
    b_d = nc.declare_dram_parameter("bias", [1, C_CORE], F32, isOutput=False)
    i_d = nc.declare_dram_parameter("ident", [BLK, BLK], BF16, isOutput=False)
    n_d = nc.declare_dram_parameter("ones", [1, BLK], F32, isOutput=False)
    o_d = nc.declare_dram_parameter("out", [128, XP_COLS], F32, isOutput=True)

    with tile.TileContext(nc) as tc, ExitStack() as ctx:
        consts = ctx.enter_context(tc.tile_pool(name="consts", bufs=1))
        x_pool = ctx.enter_context(tc.tile_pool(name="x", bufs=5))
        xt_pool = ctx.enter_context(tc.tile_pool(name="xt", bufs=4))
        out_pool = ctx.enter_context(tc.tile_pool(name="out", bufs=4))
        tp_pool = ctx.enter_context(tc.tile_pool(name="tp", bufs=3, space="PSUM"))
        mp_pool = ctx.enter_context(tc.tile_pool(name="mp", bufs=3, space="PSUM"))
        bp_pool = ctx.enter_context(tc.tile_pool(name="bp", bufs=1, space="PSUM"))

        ident = consts.tile([BLK, BLK], BF16)
        ones = consts.tile([1, BLK], F32)
        w_sb = consts.tile([BLK, C_CORE], BF16)
        b_row = consts.tile([1, C_CORE], F32)
        bias_sb = consts.tile([128, C_CORE], F32)

        # Consts: identity (needed by the first transpose ~10.5 us in)
        # leads the Sync ring; weights/bias lead the ACT ring ahead of
        # the x stream.
        nc.sync.dma_start(out=ident, in_=i_d[:, :])
        nc.sync.dma_start(out=ones, in_=n_d[:, :])
        nc.scalar.dma_start(out=w_sb, in_=w_d[:, :])
        nc.scalar.dma_start(out=b_row, in_=b_d[:, :])

        # Broadcast bias across partitions: [128,512] = ones.T @ b_row.
        bp = bp_pool.tile([128, C_CORE], F32)
        nc.tensor.matmul(bp, ones, b_row, start=True, stop=True)
        nc.vector.tensor_copy(bias_sb, bp)

        x_chunks = [None] * N_CHUNKS

        def emit_load(c):
            # x is host-cast to bf16, halving load-side HBM bytes (the
            # dominant lever: per-core HBM drops 33.8 -> 25.4 MB).  Plain
            # SWDGE (gpsimd) loads keep both HWDGE rings free for stores.
            xc = x_pool.tile([128, G_PER_CHUNK * C_CORE], BF16)
            base = c * G_PER_CHUNK * C_CORE
            if c == 0:
                # split so the first transposes start earlier
                nc.gpsimd.dma_start(out=xc[:, 0:512], in_=x_d[:, 0:512])
                nc.gpsimd.dma_start(out=xc[:, 512:2048], in_=x_d[:, 512:2048])
                nc.gpsimd.dma_start(out=xc[:, 2048:4096], in_=x_d[:, 2048:4096])
            else:
                nc.gpsimd.dma_start(out=xc, in_=x_d[:, base:base + G_PER_CHUNK * C_CORE])
            x_chunks[c] = xc

        # Plain bf16 SWDGE measures ~405 GB/s solo, so front-load deep:
        # the whole x shard is in SBUF by ~1/3 of the kernel and the
        # rings then stream stores at the full HBM rate.
        for c in range(5):
            emit_load(c)

        def emit_transposes(g):
            tp = tp_pool.tile([128, C_CORE], BF16)
            xc = x_chunks[g // G_PER_CHUNK]
            gb = (g % G_PER_CHUNK) * C_CORE
            for j in range(NB_CORE):
                nc.tensor.matmul(
                    tp[:, j * 128:(j + 1) * 128],
                    xc[:, gb + j * 128:gb + (j + 1) * 128],
                    ident,
                    is_transpose=True,
                    start=(j == 0),
                    stop=(j == NB_CORE - 1),
                )
            xt = xt_pool.tile([128, C_CORE], BF16)
            nc.scalar.copy(xt, tp)   # PSUM f32 -> SBUF bf16
            return xt

        xt_q = [emit_transposes(0), emit_transposes(1)]
        out_tile = None
        for g in range(GROUPS):
            if g % G_PER_OUT == 0:
                out_tile = out_pool.tile([128, G_PER_OUT * C_CORE], F32)
            # remaining chunks paced only by buffer reuse (WAR)
            if g % G_PER_CHUNK == 0 and 5 <= (gc := g // G_PER_CHUNK + 5) < N_CHUNKS:
                emit_load(gc)
            xt = xt_q.pop(0)
            if g + 2 < GROUPS:
                xt_q.append(emit_transposes(g + 2))
            mp = mp_pool.tile([128, C_CORE], F32)
            for j in range(NB_CORE):
                nc.tensor.matmul(
                    mp[:, j * 128:(j + 1) * 128],
                    xt[:, j * 128:(j + 1) * 128],
                    w_sb[:, j * 128:(j + 1) * 128],
                    start=(j == 0),
                    stop=(j == NB_CORE - 1),
                )
            gi = (g % G_PER_OUT) * C_CORE
            nc.vector.tensor_add(out_tile[:, gi:gi + C_CORE], mp, bias_sb)
            # Loads ride the SWDGE queue, so BOTH HWDGE rings carry
            # stores: alternate out-tiles between them (8.4 MB each).
            if g >= GROUPS - TAIL_GROUPS:
                # tail: store per-pair alternating rings so the kernel
                # tail only waits on 512 KiB.
                if g % 2 == 1:
                    eng = nc.sync if g % 4 == 1 else nc.scalar
                    cols = slice((g - 1) * C_CORE, (g + 1) * C_CORE)
                    eng.dma_start(
                        out=o_d[:, cols],
                        in_=out_tile[:, gi - C_CORE:gi + C_CORE],
                    )
            elif g % G_PER_OUT == G_PER_OUT - 1:
                t = g // G_PER_OUT
                eng = nc.sync if t % 2 == 0 else nc.scalar
                cols = slice(t * G_PER_OUT * C_CORE, (t + 1) * G_PER_OUT * C_CORE)
                eng.dma_start(out=o_d[:, cols], in_=out_tile)

    nc.compile()
    return nc


def _get_nc():
    if "nc" not in _NC_CACHE:
        _NC_CACHE["nc"] = _build_nc()
    return _NC_CACHE["nc"]


def _run(inputs, trace=False):
    x = np.asarray(inputs["x"], dtype=np.float32)
    weights = np.asarray(inputs["weights"], dtype=np.float32)
    bias = np.asarray(inputs["bias"], dtype=np.float32)
    orig_shape = x.shape
    xf = x.reshape(B_FULL, SIZE)
    ident = np.eye(BLK, dtype=np.float32).astype(ml_dtypes.bfloat16)
    ones = np.ones((1, BLK), dtype=np.float32)

    nc = _get_nc()
    in_maps = []
    for i in range(N_CORES):
        cols = slice(i * C_CORE, (i + 1) * C_CORE)
        # pack: xp[p, g*512 + c] = xf[g*128 + p, 512*i + c]
        xp = np.ascontiguousarray(
            xf[:, cols].reshape(GROUPS, 128, C_CORE).transpose(1, 0, 2)
            .reshape(128, XP_COLS)
        ).astype(ml_dtypes.bfloat16)
        # weights d-major per core: [d, j*128+e] = W[4i+j, d, e], cast bf16
        w_t = np.ascontiguousarray(
            weights[i * NB_CORE:(i + 1) * NB_CORE].transpose(1, 0, 2)
            .reshape(BLK, C_CORE)
        ).astype(ml_dtypes.bfloat16)
        in_maps.append(
            {
                "x": xp,
                "weights": w_t,
                "bias": np.ascontiguousarray(bias[cols][None, :]),
                "ident": ident,
                "ones": ones,
            }
        )
    res = run_bass_kernel_spmd(
        nc, in_maps, core_ids=list(range(N_CORES)), trace=trace
    )
    out = np.empty((B_FULL, SIZE), dtype=np.float32)
    for i in range(N_CORES):
        cols = slice(i * C_CORE, (i + 1) * C_CORE)
        op = res.results[i]["out"]
        out[:, cols] = (
            op.reshape(128, GROUPS, C_CORE).transpose(1, 0, 2)
            .reshape(B_FULL, C_CORE)
        )
    return out.reshape(orig_shape), res


def kernel(**inputs):
    out, _ = _run(inputs, trace=False)
    return out


# revision 58
# speedup vs baseline: 1.2746x; 1.0066x over previous
"""Block-diagonal MLP kernel for Trainium2 (8 NeuronCores, expert-sharded).

Computes out = blockdiag_matmul(x, weights) + bias where
  x: [4, 2048, 4096] f32, weights: [32, 128, 128] f32, bias: [4096] f32.

Strategy: shard the 32 diagonal blocks across 8 cores (4 blocks = 512
feature columns each); every core sees all 8192 flattened rows of its
512-column slice.  Measured per-core HBM read+write tops out at
~430 GB/s; a single HWDGE ring caps at ~240-340 GB/s.

The host packs each core's x shard as [128, 32768] (partition p holds
the rows congruent to p mod 128, 64 row-groups side by side) AND
pre-casts it to bf16 -- halving load-side HBM bytes, the single
biggest lever (per-core HBM drops 33.8 -> 25.4 MB, floor ~59 us).
x streams through a rotating pool of [128, 4096] bf16 chunk buffers
(8 groups each) on the plain SWDGE (gpsimd) queue (~405 GB/s solo),
front-loaded 5 chunks deep; that frees BOTH HWDGE rings for stores,
which alternate per out-tile (8.4 MB each ring) and overlap loads for
the whole body.  The last four pairs store individually alternating
across both rings so the tail only waits on 512 KiB.

Compute runs in PAIRS of 512-column groups to halve per-op overhead on
the evacuation engines: 8 PE transpose-mode matmuls (bf16) fill one
[128,1024] bf16 PSUM bank; one ACT copy evacuates the pair to SBUF;
8 bf16 matmuls against SBUF-resident bf16 weights (host-cast, the same
4 blocks throughout) fill a [128,1024] f32 PSUM tile; one DVE add
evacuates it with the (pre-doubled) bias fused.  Transposes run two
pairs ahead of the consuming matmuls.  The bias row is broadcast to
128 partitions once on-chip via K=1 ones-matmuls.  PSUM budget:
3x1-bank tp + 2x2-bank mp + 1x1-bank bias staging = 8 banks exactly.
bf16 is only used for matmul operands (fp32 PSUM accumulation); max
rel err vs the fp32 reference ~2e-3 (gate 2e-2).
"""
import numpy as np
import ml_dtypes
from contextlib import ExitStack

import concourse.mybir as mybir
import concourse.tile as tile
from concourse import bacc
from concourse.bass_utils import run_bass_kernel_spmd

F32 = mybir.dt.float32
BF16 = mybir.dt.bfloat16

SIZE = 4096
NB = 32            # number of diagonal blocks
BLK = 128          # block size
N_CORES = 8
NB_CORE = NB // N_CORES        # 4 blocks per core
C_CORE = NB_CORE * BLK         # 512 feature columns per core
B_FULL = 4 * 2048              # 8192 flattened rows (all on every core)
GROUPS = B_FULL // 128         # 64 row-groups of [128, 512]
XP_COLS = GROUPS * C_CORE      # 32768 packed columns
G_PER_CHUNK = 8                # groups per load chunk [128, 4096]
N_CHUNKS = GROUPS // G_PER_CHUNK
PW = 2 * C_CORE                # 1024 cols per compute pair
PAIRS = GROUPS // 2            # 32 pipeline stages of 2 groups each

_NC_CACHE = {}


def _build_nc():
    nc = bacc.Bacc()
    x_d = nc.declare_dram_parameter("x", [128, XP_COLS], BF16, isOutput=False)
    w_d = nc.declare_dram_parameter("weights", [BLK, C_CORE], BF16, isOutput=False)
    b_d = nc.declare_dram_parameter("bias", [1, PW], F32, isOutput=False)
    i_d = nc.declare_dram_parameter("ident", [BLK, BLK], BF16, isOutput=False)
    n_d = nc.declare_dram_parameter("ones", [1, BLK], F32, isOutput=False)
    o_d = nc.declare_dram_parameter("out", [128, XP_COLS], F32, isOutput=True)

    with tile.TileContext(nc) as tc, ExitStack() as ctx:
        consts = ctx.enter_context(tc.tile_pool(name="consts", bufs=1))
        x_pool = ctx.enter_context(tc.tile_pool(name="x", bufs=5))
        xt_pool = ctx.enter_context(tc.tile_pool(name="xt", bufs=4))
        out_pool = ctx.enter_context(tc.tile_pool(name="out", bufs=4))
        tp_pool = ctx.enter_context(tc.tile_pool(name="tp", bufs=3, space="PSUM"))
        mp_pool = ctx.enter_context(tc.tile_pool(name="mp", bufs=2, space="PSUM"))
        bp_pool = ctx.enter_context(tc.tile_pool(name="bp", bufs=1, space="PSUM"))

        ident = consts.tile([BLK, BLK], BF16)
        ones = consts.tile([1, BLK], F32)
        w_sb = consts.tile([BLK, C_CORE], BF16)
        b_row = consts.tile([1, PW], F32)
        bias_sb = consts.tile([128, PW], F32)

        # Consts: identity (needed by the first transpose) leads the
        # Sync ring; weights/bias lead the ACT ring ahead of the stores.
        nc.sync.dma_start(out=ident, in_=i_d[:, :])
        nc.sync.dma_start(out=ones, in_=n_d[:, :])
        nc.scalar.dma_start(out=w_sb, in_=w_d[:, :])
        nc.scalar.dma_start(out=b_row, in_=b_d[:, :])

        # Broadcast bias across partitions via K=1 ones-matmuls, staged
        # through a single PSUM bank in two [128,512] rounds.
        for h in range(2):
            bp = bp_pool.tile([128, C_CORE], F32)
            cols = slice(h * C_CORE, (h + 1) * C_CORE)
            nc.tensor.matmul(bp, ones, b_row[:, cols], start=True, stop=True)
            nc.vector.tensor_copy(bias_sb[:, cols], bp)

        x_chunks = [None] * N_CHUNKS

        def emit_load(c):
            # x is host-cast to bf16, halving load-side HBM bytes.  Plain
            # SWDGE (gpsimd) loads keep both HWDGE rings free for stores.
            xc = x_pool.tile([128, G_PER_CHUNK * C_CORE], BF16)
            base = c * G_PER_CHUNK * C_CORE
            if c == 0:
                # split so the first transposes start earlier
                nc.gpsimd.dma_start(out=xc[:, 0:1024], in_=x_d[:, 0:1024])
                nc.gpsimd.dma_start(out=xc[:, 1024:4096], in_=x_d[:, 1024:4096])
            else:
                nc.gpsimd.dma_start(out=xc, in_=x_d[:, base:base + G_PER_CHUNK * C_CORE])
            x_chunks[c] = xc

        # Plain bf16 SWDGE measures ~405 GB/s solo, so front-load deep:
        # the whole x shard is in SBUF by ~1/3 of the kernel and the
        # rings then stream stores at the full HBM rate.
        for c in range(5):
            emit_load(c)

        def emit_transposes(p):
            # Both groups' transposes land in one [128,1024] bf16 PSUM
            # bank; one ACT copy evacuates the pair.
            tp = tp_pool.tile([128, PW], BF16)
            xc = x_chunks[2 * p // G_PER_CHUNK]
            for j in range(8):
                g = 2 * p + j // 4
                col = (g % G_PER_CHUNK) * C_CORE + (j % 4) * 128
                nc.tensor.matmul(
                    tp[:, j * 128:(j + 1) * 128],
                    xc[:, col:col + 128],
                    ident,
                    is_transpose=True,
                    start=(j % 4 == 0),
                    stop=(j % 4 == 3),
                )
            xt = xt_pool.tile([128, PW], BF16)
            nc.scalar.copy(xt, tp)   # PSUM -> SBUF, one op per pair
            return xt

        xt_q = [emit_transposes(0), emit_transposes(1)]
        out_tile = None
        for p in range(PAIRS):
            if p % 2 == 0:
                out_tile = out_pool.tile([128, 2 * PW], F32)
            # remaining chunks paced only by buffer reuse (WAR)
            if p % 4 == 0 and 5 <= (gc := p // 4 + 5) < N_CHUNKS:
                emit_load(gc)
            xt = xt_q.pop(0)
            if p + 2 < PAIRS:
                xt_q.append(emit_transposes(p + 2))
            mp = mp_pool.tile([128, PW], F32)
            for j in range(8):
                k = j % 4
                nc.tensor.matmul(
                    mp[:, j * 128:(j + 1) * 128],
                    xt[:, j * 128:(j + 1) * 128],
                    w_sb[:, k * 128:(k + 1) * 128],
                    start=(j % 4 == 0),
                    stop=(j % 4 == 3),
                )
            gi = (p % 2) * PW
            nc.vector.tensor_add(out_tile[:, gi:gi + PW], mp, bias_sb)
            # Loads ride the SWDGE queue, so BOTH HWDGE rings carry
            # stores: alternate out-tiles between them (8.4 MB each).
            if p >= PAIRS - 4:
                # tail: store per-pair alternating rings so the kernel
                # tail only waits on 512 KiB.
                eng = nc.sync if p % 2 == 0 else nc.scalar
                cols = slice(p * PW, (p + 1) * PW)
                eng.dma_start(out=o_d[:, cols], in_=out_tile[:, gi:gi + PW])
            elif p % 2 == 1:
                t = p // 2
                eng = nc.sync if t % 2 == 0 else nc.scalar
                cols = slice(t * 2 * PW, (t + 1) * 2 * PW)
                eng.dma_start(out=o_d[:, cols], in_=out_tile)

    nc.compile()
    return nc


def _get_nc():
    if "nc" not in _NC_CACHE:
        _NC_CACHE["nc"] = _build_nc()
    return _NC_CACHE["nc"]


def _run(inputs, trace=False):
    x = np.asarray(inputs["x"], dtype=np.float32)
    weights = np.asarray(inputs["weights"], dtype=np.float32)
    bias = np.asarray(inputs["bias"], dtype=np.float32)
    orig_shape = x.shape
    xf = x.reshape(B_FULL, SIZE)
    ident32 = np.eye(BLK, dtype=np.float32)
    ident = ident32.astype(ml_dtypes.bfloat16)
    ones = np.ones((1, BLK), dtype=np.float32)

    nc = _get_nc()
    in_maps = []
    for i in range(N_CORES):
        cols = slice(i * C_CORE, (i + 1) * C_CORE)
        # pack: xp[p, g*512 + c] = xf[g*128 + p, 512*i + c], cast bf16
        xp = np.ascontiguousarray(
            xf[:, cols].reshape(GROUPS, 128, C_CORE).transpose(1, 0, 2)
            .reshape(128, XP_COLS)
        ).astype(ml_dtypes.bfloat16)
        # weights d-major per core: [d, j*128+e] = W[4i+j, d, e], cast bf16
        w_t = np.ascontiguousarray(
            weights[i * NB_CORE:(i + 1) * NB_CORE].transpose(1, 0, 2)
            .reshape(BLK, C_CORE)
        ).astype(ml_dtypes.bfloat16)
        in_maps.append(
            {
                "x": xp,
                "weights": w_t,
                "bias": np.ascontiguousarray(np.tile(bias[cols], 2)[None, :]),
                "ident": ident,
                "ones": ones,
            }
        )
    res = run_bass_kernel_spmd(
        nc, in_maps, core_ids=list(range(N_CORES)), trace=trace
    )
    out = np.empty((B_FULL, SIZE), dtype=np.float32)
    for i in range(N_CORES):
        cols = slice(i * C_CORE, (i + 1) * C_CORE)
        op = res.results[i]["out"]
        out[:, cols] = (
            op.reshape(128, GROUPS, C_CORE).transpose(1, 0, 2)
            .reshape(B_FULL, C_CORE)
        )
    return out.reshape(orig_shape), res


def kernel(**inputs):
    out, _ = _run(inputs, trace=False)
    return out
